# revision 7
# baseline (speedup 1.0000x reference)
"""Trainium2 Bass kernel for nn_ChiSquareLoss (histogram binning + chi-square).

Strategy (pure data parallel across 8 NeuronCores, 4 images/core):
  - Each core receives 24 "planes" of 512x512 fp32 pixels in [0,1):
    4 images x 3 channels x 2 input tensors, laid out as [24, 128, 2048].
  - Per plane, a 256-bin histogram factored as 16 hi x 16 lo bins, with
    CUMULATIVE (is_ge) factors instead of one-hot:
      idx = int16(255*x - 0.5)   (output-conversion rounding == floor for
            non-integer 255x; rare integer-255x ties corrected on host)
      lo  = idx & 15             (int16 bitwise_and)
      him[j] = (idx >= 16j - .5)           j=0..15   (cumulative hi factor)
      lom[i] = (lo  >= i - .5)             i=0..15   (cumulative lo factor)
    A few him rows are computed on ScalarE as sign(idx - 16j + .5) in
    {-1,+1} (one ACTIVATE pass each); sign = 2*is_ge - 1 keeps the factor
    matrix invertible, and the host solves the 16x16 system exactly.
  - Layouts: him (stationary operand) is pack-interleaved [P, npack, 16, 8]
    so each matmul lhsT slab is a contiguous 128-wide region; lom (moving
    operand) is bin-major [P, 16, fch] so every DVE mask write is a dense
    [128, fch] region (4x perf mode) -- the compiler accepts strided APs
    for the moving operand only.
  - S[j,i] = sum_pixels him[j]*lom[i] via TensorE outer-product matmuls,
    8 pixel-columns packed per [128,128] bf16 matmul accumulated in PSUM;
    the 8 stride-8 diagonal blocks hold S.
  - Host: S = U N V^T with known invertible U, V; recover true counts N,
    assemble [32, 768] histograms, finish chi-square + mean in float64.
"""

import sys

if "/opt/trn_rl_repo" not in sys.path:
    sys.path.insert(0, "/opt/trn_rl_repo")

from contextlib import ExitStack

import numpy as np

import concourse.bacc as bacc
import concourse.bass as bass
import concourse.tile as tile
from concourse import mybir
from concourse.bass_utils import run_bass_kernel_spmd

ALU = mybir.AluOpType
ACTF = mybir.ActivationFunctionType
F32 = mybir.dt.float32
BF16 = mybir.dt.bfloat16
I16 = mybir.dt.int16

B, C, H, W = 32, 3, 512, 512
NCORES = 8
IMGS = B // NCORES            # images per core
PLANES = IMGS * C * 2         # 24 planes per core (hist1 planes then hist2 planes)
P = 128                       # SBUF partitions
FREE = (H * W) // P           # 2048 pixel columns per plane
FCH = 1024                    # free-dim chunk size
NCH = FREE // FCH
PACK = 8                      # pixel columns packed per matmul
NBINS = 256
BIAS = 1e-10

N_ACT_HI = 7                  # him rows on ScalarE via one-pass Sign
N_GPS_LO = 3                  # lom rows on GPSIMD is_ge

_cache = {}


def build_kernel(planes=PLANES, free=FREE, fch=FCH, n_act=N_ACT_HI,
                 n_gps=N_GPS_LO):
    nc = bacc.Bacc()
    x_in = nc.declare_dram_parameter("x", [planes, P, free], F32, isOutput=False)
    h_out = nc.declare_dram_parameter("h", [planes, P, P], F32, isOutput=True)

    nch = free // fch
    npack = fch // PACK

    with ExitStack() as ctx:
        tc = ctx.enter_context(tile.TileContext(nc))
        const_pool = ctx.enter_context(tc.tile_pool(name="const", bufs=1))
        pix_pool = ctx.enter_context(tc.tile_pool(name="pix", bufs=3))
        tmp_pool = ctx.enter_context(tc.tile_pool(name="tmp", bufs=2))
        mask_pool = ctx.enter_context(tc.tile_pool(name="mask", bufs=2))
        psum_pool = ctx.enter_context(tc.tile_pool(name="ps", bufs=8, space="PSUM"))
        out_pool = ctx.enter_context(tc.tile_pool(name="hout", bufs=4))

        sign_bias = {}
        for j in range(16 - n_act, 16):
            t = const_pool.tile([P, 1], F32, tag=f"sb{j}")
            nc.vector.memset(t, 0.5 - 16.0 * j)
            sign_bias[j] = t

        for pl in range(planes):
            ps = psum_pool.tile([P, P], F32, tag="ps")
            for ch in range(nch):
                x_t = pix_pool.tile([P, fch], F32, tag="x")
                nc.sync.dma_start(out=x_t, in_=x_in[pl, :, ch * fch:(ch + 1) * fch])

                # idx = int16(255x - 0.5): the int output conversion lands on
                # floor(255x) for non-integer 255x (ties: host-corrected).
                idx = tmp_pool.tile([P, fch], I16, tag="idx")
                nc.vector.tensor_scalar(idx, x_t, 255.0, -0.5, ALU.mult, ALU.add)
                lo = tmp_pool.tile([P, fch], I16, tag="lo")
                nc.vector.tensor_scalar(lo, idx, 15, None, ALU.bitwise_and)

                # him: stationary operand, pack-interleaved (contiguous lhsT
                # slabs). DVE writes are 8-elem runs; ScalarE takes n_act rows
                # as one-pass Sign (exact +-1).
                him = mask_pool.tile([P, npack, 16, PACK], BF16, tag="him")
                idx_r = idx.rearrange("p (s t) -> p s t", t=PACK)
                for j in range(16 - n_act):
                    nc.vector.tensor_scalar(
                        him[:, :, j, :], idx_r, 16.0 * j - 0.5, None, ALU.is_ge
                    )
                for j in range(16 - n_act, 16):
                    nc.scalar.activation(
                        him[:, :, j, :], idx_r, ACTF.Sign,
                        bias=sign_bias[j][:, 0:1], scale=1.0,
                    )

                # lom: moving operand, bin-major (dense [128, fch] writes,
                # DVE 4x perf mode); strided rhs APs are accepted. A few rows
                # go to the otherwise-idle GPSIMD.
                lom = mask_pool.tile([P, 16, fch], BF16, tag="lom")
                for i in range(16):
                    eng = nc.gpsimd if i >= 16 - n_gps else nc.vector
                    eng.tensor_scalar(
                        lom[:, i, :], lo, i - 0.5, None, ALU.is_ge
                    )

                for s in range(npack):
                    lhsT = him[:, s].rearrange("p j t -> p (j t)")
                    rhs = lom[:, :, s * PACK:(s + 1) * PACK]
                    nc.tensor.matmul(
                        ps,
                        lhsT,
                        rhs,
                        start=(ch == 0 and s == 0),
                        stop=(ch == nch - 1 and s == npack - 1),
                    )

            hist_sb = out_pool.tile([P, P], F32, tag="hist")
            nc.vector.tensor_copy(hist_sb, ps)
            nc.sync.dma_start(out=h_out[pl], in_=hist_sb)

    nc.finalize()
    return nc


def _get_nc():
    if "nc" not in _cache:
        _cache["nc"] = build_kernel()
    return _cache["nc"]


def shard_inputs(hist1: np.ndarray, hist2: np.ndarray):
    """Build per-core input maps: core i gets images [4i, 4i+4) of both tensors."""
    in_maps = []
    for i in range(NCORES):
        sl1 = hist1[i * IMGS:(i + 1) * IMGS]  # [4, 3, 512, 512]
        sl2 = hist2[i * IMGS:(i + 1) * IMGS]
        x = np.concatenate(
            [
                np.ascontiguousarray(sl1).reshape(IMGS * C, P, FREE),
                np.ascontiguousarray(sl2).reshape(IMGS * C, P, FREE),
            ],
            axis=0,
        )  # [24, 128, 2048]
        in_maps.append({"x": np.ascontiguousarray(x, dtype=np.float32)})
    return in_maps


def _recovery_mats(n_act=N_ACT_HI):
    """U (him rows) and V (lom rows): S = U N V^T -> N = Uinv S Vinv^T.

    Cumulative rows U[j,a] = 1[a>=j]; ScalarE rows are sign-form 2*1[a>=j]-1.
    """
    a = np.arange(16)
    U = (a[None, :] >= np.arange(16)[:, None]).astype(np.float64)
    U[16 - n_act:, :] = 2.0 * U[16 - n_act:, :] - 1.0
    V = (a[None, :] >= np.arange(16)[:, None]).astype(np.float64)
    return np.linalg.inv(U), np.linalg.inv(V)


_UINV, _VINV = _recovery_mats()


def hist2d_from_raw(raw: np.ndarray) -> np.ndarray:
    """raw: [..., 128, 128] PSUM accumulators -> [..., 256] histograms.

    PSUM row m = 8*j + t, col n = 8*i + t'; valid sums live on the t == t'
    diagonals: S[j, i] = sum_t raw[8j+t, 8i+t].  Then N = Uinv S Vinv^T.
    """
    lead = raw.shape[:-2]
    r = raw.reshape(lead + (16, PACK, 16, PACK)).astype(np.float64)
    S = np.einsum("...jtit->...ji", r)
    N = np.einsum("ja,...ab,ib->...ji", _UINV, S, _VINV)
    N = np.rint(N)
    return N.reshape(lead + (NBINS,))


def fixup_hist(hist: np.ndarray, plane_x: np.ndarray) -> None:
    """Correct the RNE tie cases in-place so counts match exact floor binning.

    Device semantics: for z = fl(255*x) exactly an odd integer k, the RNE tie
    binned the pixel at k-1 instead of k. All other pixels are binned exactly.
    """
    z = plane_x.astype(np.float32) * np.float32(255.0)
    zf = z[z == np.floor(z)]
    if zf.size == 0:
        return
    k = zf.astype(np.int64)
    odd = k[k % 2 == 1]
    for kk, cnt in zip(*np.unique(odd, return_counts=True)):
        hist[kk - 1] -= cnt
        hist[kk] += cnt


def finish_on_host(per_core_hists: list) -> np.ndarray:
    """per_core_hists: NCORES arrays [24, 256] -> scalar chi-square loss."""
    h = np.stack(per_core_hists)  # [8, 24, 256]
    h = h.reshape(NCORES, 2, IMGS, C, NBINS)
    counts1 = h[:, 0].reshape(B, C * NBINS)  # [32, 768]
    counts2 = h[:, 1].reshape(B, C * NBINS)
    n = float(C * H * W)
    h1 = counts1 / n
    h2 = counts2 / n
    chi = np.sum((h1 - h2) ** 2 / (h1 + h2 + BIAS), axis=1)
    return np.array(np.mean(chi), dtype=np.float32)


def kernel(hist1: np.ndarray, hist2: np.ndarray) -> np.ndarray:
    hist1 = np.asarray(hist1, dtype=np.float32)
    hist2 = np.asarray(hist2, dtype=np.float32)
    nc = _get_nc()
    in_maps = shard_inputs(hist1, hist2)
    res = run_bass_kernel_spmd(nc, in_maps, list(range(NCORES)))
    per_core = []
    for i in range(NCORES):
        hists = hist2d_from_raw(res.results[i]["h"])  # [24, 256]
        for pl in range(PLANES):
            fixup_hist(hists[pl], in_maps[i]["x"][pl])
        per_core.append(hists)
    return finish_on_host(per_core)


if __name__ == "__main__":
    rng = np.random.default_rng(0)
    h1 = rng.random((B, C, H, W), dtype=np.float32)
    h2 = rng.random((B, C, H, W), dtype=np.float32)
    out = kernel(h1, h2)
    print("kernel output:", out)


# revision 8
# speedup vs baseline: 5.1348x; 5.1348x over previous
"""Trainium2 Bass kernel for nn_ChiSquareLoss (histogram binning + chi-square).

Strategy (pure data parallel across 8 NeuronCores, 4 images/core):
  - Each core receives 24 "planes" of 512x512 fp32 pixels in [0,1):
    4 images x 3 channels x 2 input tensors, laid out as [24, 128, 2048].
  - Per plane, a 256-bin histogram factored as 16 hi x 16 lo bins, with
    CUMULATIVE (is_ge) factors instead of one-hot:
      idx = int16(255*x - 0.5)   (output-conversion rounding == floor for
            non-integer 255x; rare integer-255x ties corrected on host)
      lo  = idx & 15             (int16 bitwise_and)
      him[j] = (idx >= 16j - .5)           j=0..15   (cumulative hi factor)
      lom[i] = (lo  >= i - .5)             i=0..15   (cumulative lo factor)
    A few him rows are computed on ScalarE as sign(idx - 16j + .5) in
    {-1,+1} (one ACTIVATE pass each); sign = 2*is_ge - 1 keeps the factor
    matrix invertible, and the host solves the 16x16 system exactly.
  - Layouts: him (stationary operand) is pack-interleaved [P, npack, 16, 8]
    so each matmul lhsT slab is a contiguous 128-wide region; lom (moving
    operand) is bin-major [P, 16, fch] so every DVE mask write is a dense
    [128, fch] region (4x perf mode) -- the compiler accepts strided APs
    for the moving operand only.
  - S[j,i] = sum_pixels him[j]*lom[i] via TensorE outer-product matmuls,
    8 pixel-columns packed per [128,128] bf16 matmul accumulated in PSUM;
    the 8 stride-8 diagonal blocks hold S.
  - Host: S = U N V^T with known invertible U, V; recover true counts N,
    assemble [32, 768] histograms, finish chi-square + mean in float64.
"""

import sys

if "/opt/trn_rl_repo" not in sys.path:
    sys.path.insert(0, "/opt/trn_rl_repo")

from contextlib import ExitStack

import numpy as np

import concourse.bacc as bacc
import concourse.bass as bass
import concourse.tile as tile
from concourse import mybir
from concourse.bass_utils import run_bass_kernel_spmd

ALU = mybir.AluOpType
ACTF = mybir.ActivationFunctionType
F32 = mybir.dt.float32
BF16 = mybir.dt.bfloat16
I16 = mybir.dt.int16

B, C, H, W = 32, 3, 512, 512
NCORES = 8
IMGS = B // NCORES            # images per core
PLANES = IMGS * C * 2         # 24 planes per core (hist1 planes then hist2 planes)
P = 128                       # SBUF partitions
FREE = (H * W) // P           # 2048 pixel columns per plane
FCH = 1024                    # free-dim chunk size
NCH = FREE // FCH
PACK = 8                      # pixel columns packed per matmul
NBINS = 256
BIAS = 1e-10

N_ACT_HI = 7                  # him rows on ScalarE via one-pass Sign
N_GPS_LO = 0                  # lom rows on GPSIMD is_ge (port-shared with DVE:
                              # n_gps=3 measured 5x WORSE, 2.6ms -- keep 0)

_cache = {}


def build_kernel(planes=PLANES, free=FREE, fch=FCH, n_act=N_ACT_HI,
                 n_gps=N_GPS_LO):
    nc = bacc.Bacc()
    x_in = nc.declare_dram_parameter("x", [planes, P, free], F32, isOutput=False)
    h_out = nc.declare_dram_parameter("h", [planes, P, P], F32, isOutput=True)

    nch = free // fch
    npack = fch // PACK

    with ExitStack() as ctx:
        tc = ctx.enter_context(tile.TileContext(nc))
        const_pool = ctx.enter_context(tc.tile_pool(name="const", bufs=1))
        pix_pool = ctx.enter_context(tc.tile_pool(name="pix", bufs=3))
        tmp_pool = ctx.enter_context(tc.tile_pool(name="tmp", bufs=2))
        mask_pool = ctx.enter_context(tc.tile_pool(name="mask", bufs=2))
        psum_pool = ctx.enter_context(tc.tile_pool(name="ps", bufs=8, space="PSUM"))
        out_pool = ctx.enter_context(tc.tile_pool(name="hout", bufs=4))

        sign_bias = {}
        for j in range(16 - n_act, 16):
            t = const_pool.tile([P, 1], F32, tag=f"sb{j}")
            nc.vector.memset(t, 0.5 - 16.0 * j)
            sign_bias[j] = t

        for pl in range(planes):
            ps = psum_pool.tile([P, P], F32, tag="ps")
            for ch in range(nch):
                x_t = pix_pool.tile([P, fch], F32, tag="x")
                nc.sync.dma_start(out=x_t, in_=x_in[pl, :, ch * fch:(ch + 1) * fch])

                # idx = int16(255x - 0.5): the int output conversion lands on
                # floor(255x) for non-integer 255x (ties: host-corrected).
                idx = tmp_pool.tile([P, fch], I16, tag="idx")
                nc.vector.tensor_scalar(idx, x_t, 255.0, -0.5, ALU.mult, ALU.add)
                lo = tmp_pool.tile([P, fch], I16, tag="lo")
                nc.vector.tensor_scalar(lo, idx, 15, None, ALU.bitwise_and)

                # him: stationary operand, pack-interleaved (contiguous lhsT
                # slabs). DVE writes are 8-elem runs; ScalarE takes n_act rows
                # as one-pass Sign (exact +-1).
                him = mask_pool.tile([P, npack, 16, PACK], BF16, tag="him")
                idx_r = idx.rearrange("p (s t) -> p s t", t=PACK)
                for j in range(16 - n_act):
                    nc.vector.tensor_scalar(
                        him[:, :, j, :], idx_r, 16.0 * j - 0.5, None, ALU.is_ge
                    )
                for j in range(16 - n_act, 16):
                    nc.scalar.activation(
                        him[:, :, j, :], idx_r, ACTF.Sign,
                        bias=sign_bias[j][:, 0:1], scale=1.0,
                    )

                # lom: moving operand, bin-major (dense [128, fch] writes,
                # DVE 4x perf mode); strided rhs APs are accepted. A few rows
                # go to the otherwise-idle GPSIMD.
                lom = mask_pool.tile([P, 16, fch], BF16, tag="lom")
                for i in range(16):
                    eng = nc.gpsimd if i >= 16 - n_gps else nc.vector
                    eng.tensor_scalar(
                        lom[:, i, :], lo, i - 0.5, None, ALU.is_ge
                    )

                for s in range(npack):
                    lhsT = him[:, s].rearrange("p j t -> p (j t)")
                    rhs = lom[:, :, s * PACK:(s + 1) * PACK]
                    nc.tensor.matmul(
                        ps,
                        lhsT,
                        rhs,
                        start=(ch == 0 and s == 0),
                        stop=(ch == nch - 1 and s == npack - 1),
                    )

            hist_sb = out_pool.tile([P, P], F32, tag="hist")
            nc.vector.tensor_copy(hist_sb, ps)
            nc.sync.dma_start(out=h_out[pl], in_=hist_sb)

    nc.finalize()
    return nc


def _get_nc():
    if "nc" not in _cache:
        _cache["nc"] = build_kernel()
    return _cache["nc"]


def shard_inputs(hist1: np.ndarray, hist2: np.ndarray):
    """Build per-core input maps: core i gets images [4i, 4i+4) of both tensors."""
    in_maps = []
    for i in range(NCORES):
        sl1 = hist1[i * IMGS:(i + 1) * IMGS]  # [4, 3, 512, 512]
        sl2 = hist2[i * IMGS:(i + 1) * IMGS]
        x = np.concatenate(
            [
                np.ascontiguousarray(sl1).reshape(IMGS * C, P, FREE),
                np.ascontiguousarray(sl2).reshape(IMGS * C, P, FREE),
            ],
            axis=0,
        )  # [24, 128, 2048]
        in_maps.append({"x": np.ascontiguousarray(x, dtype=np.float32)})
    return in_maps


def _recovery_mats(n_act=N_ACT_HI):
    """U (him rows) and V (lom rows): S = U N V^T -> N = Uinv S Vinv^T.

    Cumulative rows U[j,a] = 1[a>=j]; ScalarE rows are sign-form 2*1[a>=j]-1.
    """
    a = np.arange(16)
    U = (a[None, :] >= np.arange(16)[:, None]).astype(np.float64)
    U[16 - n_act:, :] = 2.0 * U[16 - n_act:, :] - 1.0
    V = (a[None, :] >= np.arange(16)[:, None]).astype(np.float64)
    return np.linalg.inv(U), np.linalg.inv(V)


_UINV, _VINV = _recovery_mats()


def hist2d_from_raw(raw: np.ndarray) -> np.ndarray:
    """raw: [..., 128, 128] PSUM accumulators -> [..., 256] histograms.

    PSUM row m = 8*j + t, col n = 8*i + t'; valid sums live on the t == t'
    diagonals: S[j, i] = sum_t raw[8j+t, 8i+t].  Then N = Uinv S Vinv^T.
    """
    lead = raw.shape[:-2]
    r = raw.reshape(lead + (16, PACK, 16, PACK)).astype(np.float64)
    S = np.einsum("...jtit->...ji", r)
    N = np.einsum("ja,...ab,ib->...ji", _UINV, S, _VINV)
    N = np.rint(N)
    return N.reshape(lead + (NBINS,))


def fixup_hist(hist: np.ndarray, plane_x: np.ndarray) -> None:
    """Correct the RNE tie cases in-place so counts match exact floor binning.

    Device semantics: for z = fl(255*x) exactly an odd integer k, the RNE tie
    binned the pixel at k-1 instead of k. All other pixels are binned exactly.
    """
    z = plane_x.astype(np.float32) * np.float32(255.0)
    zf = z[z == np.floor(z)]
    if zf.size == 0:
        return
    k = zf.astype(np.int64)
    odd = k[k % 2 == 1]
    for kk, cnt in zip(*np.unique(odd, return_counts=True)):
        hist[kk - 1] -= cnt
        hist[kk] += cnt


def finish_on_host(per_core_hists: list) -> np.ndarray:
    """per_core_hists: NCORES arrays [24, 256] -> scalar chi-square loss."""
    h = np.stack(per_core_hists)  # [8, 24, 256]
    h = h.reshape(NCORES, 2, IMGS, C, NBINS)
    counts1 = h[:, 0].reshape(B, C * NBINS)  # [32, 768]
    counts2 = h[:, 1].reshape(B, C * NBINS)
    n = float(C * H * W)
    h1 = counts1 / n
    h2 = counts2 / n
    chi = np.sum((h1 - h2) ** 2 / (h1 + h2 + BIAS), axis=1)
    return np.array(np.mean(chi), dtype=np.float32)


def kernel(hist1: np.ndarray, hist2: np.ndarray) -> np.ndarray:
    hist1 = np.asarray(hist1, dtype=np.float32)
    hist2 = np.asarray(hist2, dtype=np.float32)
    nc = _get_nc()
    in_maps = shard_inputs(hist1, hist2)
    res = run_bass_kernel_spmd(nc, in_maps, list(range(NCORES)))
    per_core = []
    for i in range(NCORES):
        hists = hist2d_from_raw(res.results[i]["h"])  # [24, 256]
        for pl in range(PLANES):
            fixup_hist(hists[pl], in_maps[i]["x"][pl])
        per_core.append(hists)
    return finish_on_host(per_core)


if __name__ == "__main__":
    rng = np.random.default_rng(0)
    h1 = rng.random((B, C, H, W), dtype=np.float32)
    h2 = rng.random((B, C, H, W), dtype=np.float32)
    out = kernel(h1, h2)
    print("kernel output:", out)


# revision 10
# speedup vs baseline: 5.7750x; 1.1247x over previous
"""Trainium2 Bass kernel for nn_ChiSquareLoss (histogram binning + chi-square).

Strategy (pure data parallel across 8 NeuronCores, 4 images/core):
  - Each core receives 24 "planes" of 512x512 fp32 pixels in [0,1):
    4 images x 3 channels x 2 input tensors, laid out as [24, 128, 2048].
  - Per plane, a 256-bin histogram factored as 16 hi x 16 lo bins, with
    CUMULATIVE (is_ge) factors instead of one-hot:
      idx = int16(255*x - 0.5)   (output-conversion rounding == floor for
            non-integer 255x; rare integer-255x ties corrected on host)
      lo  = idx & 15             (int16 bitwise_and)
      him[j] = (idx >= 16j - .5)           j=0..15   (cumulative hi factor)
      lom[i] = (lo  >= i - .5)             i=0..15   (cumulative lo factor)
    A few him rows are computed on ScalarE as sign(idx - 16j + .5) in
    {-1,+1} (one ACTIVATE pass each); sign = 2*is_ge - 1 keeps the factor
    matrix invertible, and the host solves the 16x16 system exactly.
  - Layouts: him (stationary operand) is pack-interleaved [P, npack, 16, 8]
    so each matmul lhsT slab is a contiguous 128-wide region; lom (moving
    operand) is bin-major [P, 16, fch] so every DVE mask write is a dense
    [128, fch] region (4x perf mode) -- the compiler accepts strided APs
    for the moving operand only.
  - S[j,i] = sum_pixels him[j]*lom[i] via TensorE outer-product matmuls,
    8 pixel-columns packed per [128,128] bf16 matmul accumulated in PSUM;
    the 8 stride-8 diagonal blocks hold S.
  - Host: S = U N V^T with known invertible U, V; recover true counts N,
    assemble [32, 768] histograms, finish chi-square + mean in float64.
"""

import sys

if "/opt/trn_rl_repo" not in sys.path:
    sys.path.insert(0, "/opt/trn_rl_repo")

from contextlib import ExitStack

import numpy as np

import concourse.bacc as bacc
import concourse.bass as bass
import concourse.tile as tile
from concourse import mybir
from concourse.bass_utils import run_bass_kernel_spmd

ALU = mybir.AluOpType
ACTF = mybir.ActivationFunctionType
F32 = mybir.dt.float32
BF16 = mybir.dt.bfloat16
I16 = mybir.dt.int16

B, C, H, W = 32, 3, 512, 512
NCORES = 8
IMGS = B // NCORES            # images per core
PLANES = IMGS * C * 2         # 24 planes per core (hist1 planes then hist2 planes)
P = 128                       # SBUF partitions
FREE = (H * W) // P           # 2048 pixel columns per plane
FCH = 1024                    # free-dim chunk size
NCH = FREE // FCH
PACK = 8                      # pixel columns packed per matmul
NBINS = 256
BIAS = 1e-10

N_ACT_HI = 8                  # him rows on ScalarE via one-pass Sign
N_GPS_LO = 0                  # lom rows on GPSIMD is_ge (port-shared with DVE:
                              # n_gps=3 measured 5x WORSE, 2.6ms -- keep 0)

_cache = {}


def build_kernel(planes=PLANES, free=FREE, fch=FCH, n_act=N_ACT_HI,
                 n_gps=N_GPS_LO):
    nc = bacc.Bacc()
    x_in = nc.declare_dram_parameter("x", [planes, P, free], F32, isOutput=False)
    h_out = nc.declare_dram_parameter("h", [planes, P, P], F32, isOutput=True)

    nch = free // fch
    npack = fch // PACK

    with ExitStack() as ctx:
        tc = ctx.enter_context(tile.TileContext(nc))
        const_pool = ctx.enter_context(tc.tile_pool(name="const", bufs=1))
        pix_pool = ctx.enter_context(tc.tile_pool(name="pix", bufs=3))
        tmp_pool = ctx.enter_context(tc.tile_pool(name="tmp", bufs=2))
        mask_pool = ctx.enter_context(tc.tile_pool(name="mask", bufs=2))
        psum_pool = ctx.enter_context(tc.tile_pool(name="ps", bufs=8, space="PSUM"))
        out_pool = ctx.enter_context(tc.tile_pool(name="hout", bufs=4))

        sign_bias = {}
        for j in range(16 - n_act, 16):
            t = const_pool.tile([P, 1], F32, tag=f"sb{j}")
            nc.vector.memset(t, 0.5 - 16.0 * j)
            sign_bias[j] = t

        for pl in range(planes):
            ps = psum_pool.tile([P, P], F32, tag="ps")
            for ch in range(nch):
                x_t = pix_pool.tile([P, fch], F32, tag="x")
                nc.sync.dma_start(out=x_t, in_=x_in[pl, :, ch * fch:(ch + 1) * fch])

                # idx = int16(255x - 0.5): the int output conversion lands on
                # floor(255x) for non-integer 255x (ties: host-corrected).
                idx = tmp_pool.tile([P, fch], I16, tag="idx")
                nc.vector.tensor_scalar(idx, x_t, 255.0, -0.5, ALU.mult, ALU.add)
                lo = tmp_pool.tile([P, fch], I16, tag="lo")
                nc.vector.tensor_scalar(lo, idx, 15, None, ALU.bitwise_and)

                # him: stationary operand, pack-interleaved (contiguous lhsT
                # slabs). DVE writes are 8-elem runs; ScalarE takes n_act rows
                # as one-pass Sign (exact +-1).
                him = mask_pool.tile([P, npack, 16, PACK], BF16, tag="him")
                idx_r = idx.rearrange("p (s t) -> p s t", t=PACK)
                for j in range(16 - n_act):
                    nc.vector.tensor_scalar(
                        him[:, :, j, :], idx_r, 16.0 * j - 0.5, None, ALU.is_ge
                    )
                for j in range(16 - n_act, 16):
                    nc.scalar.activation(
                        him[:, :, j, :], idx_r, ACTF.Sign,
                        bias=sign_bias[j][:, 0:1], scale=1.0,
                    )

                # lom: moving operand, bin-major (dense [128, fch] writes,
                # DVE 4x perf mode); strided rhs APs are accepted. A few rows
                # go to the otherwise-idle GPSIMD.
                lom = mask_pool.tile([P, 16, fch], BF16, tag="lom")
                for i in range(16):
                    eng = nc.gpsimd if i >= 16 - n_gps else nc.vector
                    eng.tensor_scalar(
                        lom[:, i, :], lo, i - 0.5, None, ALU.is_ge
                    )

                for s in range(npack):
                    lhsT = him[:, s].rearrange("p j t -> p (j t)")
                    rhs = lom[:, :, s * PACK:(s + 1) * PACK]
                    nc.tensor.matmul(
                        ps,
                        lhsT,
                        rhs,
                        start=(ch == 0 and s == 0),
                        stop=(ch == nch - 1 and s == npack - 1),
                    )

            # PSUM->SBUF on ScalarE: the copy waits ~1.6us for the plane's
            # matmul chain to drain; keep that stall off the DVE critical path.
            hist_sb = out_pool.tile([P, P], F32, tag="hist")
            nc.scalar.activation(hist_sb, ps, ACTF.Copy)
            nc.sync.dma_start(out=h_out[pl], in_=hist_sb)

    nc.finalize()
    return nc


def _get_nc():
    if "nc" not in _cache:
        _cache["nc"] = build_kernel()
    return _cache["nc"]


def shard_inputs(hist1: np.ndarray, hist2: np.ndarray):
    """Build per-core input maps: core i gets images [4i, 4i+4) of both tensors."""
    in_maps = []
    for i in range(NCORES):
        sl1 = hist1[i * IMGS:(i + 1) * IMGS]  # [4, 3, 512, 512]
        sl2 = hist2[i * IMGS:(i + 1) * IMGS]
        x = np.concatenate(
            [
                np.ascontiguousarray(sl1).reshape(IMGS * C, P, FREE),
                np.ascontiguousarray(sl2).reshape(IMGS * C, P, FREE),
            ],
            axis=0,
        )  # [24, 128, 2048]
        in_maps.append({"x": np.ascontiguousarray(x, dtype=np.float32)})
    return in_maps


def _recovery_mats(n_act=N_ACT_HI):
    """U (him rows) and V (lom rows): S = U N V^T -> N = Uinv S Vinv^T.

    Cumulative rows U[j,a] = 1[a>=j]; ScalarE rows are sign-form 2*1[a>=j]-1.
    """
    a = np.arange(16)
    U = (a[None, :] >= np.arange(16)[:, None]).astype(np.float64)
    U[16 - n_act:, :] = 2.0 * U[16 - n_act:, :] - 1.0
    V = (a[None, :] >= np.arange(16)[:, None]).astype(np.float64)
    return np.linalg.inv(U), np.linalg.inv(V)


_UINV, _VINV = _recovery_mats()


def hist2d_from_raw(raw: np.ndarray) -> np.ndarray:
    """raw: [..., 128, 128] PSUM accumulators -> [..., 256] histograms.

    PSUM row m = 8*j + t, col n = 8*i + t'; valid sums live on the t == t'
    diagonals: S[j, i] = sum_t raw[8j+t, 8i+t].  Then N = Uinv S Vinv^T.
    """
    lead = raw.shape[:-2]
    r = raw.reshape(lead + (16, PACK, 16, PACK)).astype(np.float64)
    S = np.einsum("...jtit->...ji", r)
    N = np.einsum("ja,...ab,ib->...ji", _UINV, S, _VINV)
    N = np.rint(N)
    return N.reshape(lead + (NBINS,))


def fixup_hist(hist: np.ndarray, plane_x: np.ndarray) -> None:
    """Correct the RNE tie cases in-place so counts match exact floor binning.

    Device semantics: for z = fl(255*x) exactly an odd integer k, the RNE tie
    binned the pixel at k-1 instead of k. All other pixels are binned exactly.
    """
    z = plane_x.astype(np.float32) * np.float32(255.0)
    zf = z[z == np.floor(z)]
    if zf.size == 0:
        return
    k = zf.astype(np.int64)
    odd = k[k % 2 == 1]
    for kk, cnt in zip(*np.unique(odd, return_counts=True)):
        hist[kk - 1] -= cnt
        hist[kk] += cnt


def finish_on_host(per_core_hists: list) -> np.ndarray:
    """per_core_hists: NCORES arrays [24, 256] -> scalar chi-square loss."""
    h = np.stack(per_core_hists)  # [8, 24, 256]
    h = h.reshape(NCORES, 2, IMGS, C, NBINS)
    counts1 = h[:, 0].reshape(B, C * NBINS)  # [32, 768]
    counts2 = h[:, 1].reshape(B, C * NBINS)
    n = float(C * H * W)
    h1 = counts1 / n
    h2 = counts2 / n
    chi = np.sum((h1 - h2) ** 2 / (h1 + h2 + BIAS), axis=1)
    return np.array(np.mean(chi), dtype=np.float32)


def kernel(hist1: np.ndarray, hist2: np.ndarray) -> np.ndarray:
    hist1 = np.asarray(hist1, dtype=np.float32)
    hist2 = np.asarray(hist2, dtype=np.float32)
    nc = _get_nc()
    in_maps = shard_inputs(hist1, hist2)
    res = run_bass_kernel_spmd(nc, in_maps, list(range(NCORES)))
    per_core = []
    for i in range(NCORES):
        hists = hist2d_from_raw(res.results[i]["h"])  # [24, 256]
        for pl in range(PLANES):
            fixup_hist(hists[pl], in_maps[i]["x"][pl])
        per_core.append(hists)
    return finish_on_host(per_core)


if __name__ == "__main__":
    rng = np.random.default_rng(0)
    h1 = rng.random((B, C, H, W), dtype=np.float32)
    h2 = rng.random((B, C, H, W), dtype=np.float32)
    out = kernel(h1, h2)
    print("kernel output:", out)


# revision 19
# speedup vs baseline: 5.9874x; 1.0368x over previous
"""Trainium2 Bass kernel for nn_ChiSquareLoss (histogram binning + chi-square).

Strategy (pure data parallel across 8 NeuronCores, 4 images/core):
  - Each core receives 24 "planes" of 512x512 fp32 pixels in [0,1):
    4 images x 3 channels x 2 input tensors, laid out as [24, 128, 2048].
  - Per plane, a 256-bin histogram factored as 16 hi x 16 lo bins, with
    CUMULATIVE (is_ge) factors instead of one-hot:
      idx = int16(255*x - 0.5)   (output-conversion rounding == floor for
            non-integer 255x; rare integer-255x ties corrected on host)
      lo  = idx & 15             (int16 bitwise_and)
      him[j] = (idx >= 16j - .5)           j=0..15   (cumulative hi factor)
      lom[i] = (lo  >= i - .5)             i=0..15   (cumulative lo factor)
    A few him rows are computed on ScalarE as sign(idx - 16j + .5) in
    {-1,+1} (one ACTIVATE pass each); sign = 2*is_ge - 1 keeps the factor
    matrix invertible, and the host solves the 16x16 system exactly.
  - Layouts: him (stationary operand) is pack-interleaved [P, npack, 16, 8]
    so each matmul lhsT slab is a contiguous 128-wide region; lom (moving
    operand) is bin-major [P, 16, fch] so every DVE mask write is a dense
    [128, fch] region (4x perf mode) -- the compiler accepts strided APs
    for the moving operand only.
  - S[j,i] = sum_pixels him[j]*lom[i] via TensorE outer-product matmuls,
    8 pixel-columns packed per [128,128] bf16 matmul accumulated in PSUM;
    the 8 stride-8 diagonal blocks hold S.
  - Host: S = U N V^T with known invertible U, V; recover true counts N,
    assemble [32, 768] histograms, finish chi-square + mean in float64.
"""

import sys

if "/opt/trn_rl_repo" not in sys.path:
    sys.path.insert(0, "/opt/trn_rl_repo")

from contextlib import ExitStack

import numpy as np

import concourse.bacc as bacc
import concourse.bass as bass
import concourse.tile as tile
from concourse import mybir
from concourse.bass_utils import run_bass_kernel_spmd

ALU = mybir.AluOpType
ACTF = mybir.ActivationFunctionType
F32 = mybir.dt.float32
BF16 = mybir.dt.bfloat16
I16 = mybir.dt.int16

B, C, H, W = 32, 3, 512, 512
NCORES = 8
IMGS = B // NCORES            # images per core
PLANES = IMGS * C * 2         # 24 planes per core (hist1 planes then hist2 planes)
P = 128                       # SBUF partitions
FREE = (H * W) // P           # 2048 pixel columns per plane
FCH = 1024                    # free-dim chunk size
NCH = FREE // FCH
PACK = 8                      # pixel columns packed per matmul
NBINS = 256
BIAS = 1e-10

N_ACT_HI = 7                  # him rows on ScalarE via one-pass Sign
N_GPS_LO = 0                  # lom rows on GPSIMD is_ge (port-shared with DVE:
                              # n_gps=3 measured 5x WORSE, 2.6ms -- keep 0)

_cache = {}


def build_kernel(planes=PLANES, free=FREE, fch=FCH, n_act=N_ACT_HI,
                 n_gps=N_GPS_LO):
    nc = bacc.Bacc()
    x_in = nc.declare_dram_parameter("x", [planes, P, free], F32, isOutput=False)
    h_out = nc.declare_dram_parameter("h", [planes, P, P], F32, isOutput=True)

    nch = free // fch
    npack = fch // PACK

    with ExitStack() as ctx:
        tc = ctx.enter_context(tile.TileContext(nc))
        const_pool = ctx.enter_context(tc.tile_pool(name="const", bufs=1))
        pix_pool = ctx.enter_context(tc.tile_pool(name="pix", bufs=3))
        tmp_pool = ctx.enter_context(tc.tile_pool(name="tmp", bufs=2))
        mask_pool = ctx.enter_context(tc.tile_pool(name="mask", bufs=1))
        psum_pool = ctx.enter_context(tc.tile_pool(name="ps", bufs=8, space="PSUM"))
        out_pool = ctx.enter_context(tc.tile_pool(name="hout", bufs=4))

        sign_bias = {}
        for j in range(16 - n_act, 16):
            t = const_pool.tile([P, 1], F32, tag=f"sb{j}")
            nc.vector.memset(t, 0.5 - 16.0 * j)
            sign_bias[j] = t

        # Manually ping-ponged mask buffers. Row 0 of each factor is the
        # all-ones cumulative row (is_ge threshold below range): its value
        # never changes, so it is written once here and never rewritten --
        # saves 2 DVE mask ops per chunk.
        him_ab, lom_ab = [], []
        for h in range(2):
            him_t = mask_pool.tile([P, npack, 16, PACK], BF16, tag=f"him{h}")
            lom_t = mask_pool.tile([P, 16, fch], BF16, tag=f"lom{h}")
            nc.vector.memset(him_t[:, :, 0, :], 1.0)
            nc.vector.memset(lom_t[:, 0, :], 1.0)
            him_ab.append(him_t)
            lom_ab.append(lom_t)

        for pl in range(planes):
            ps = psum_pool.tile([P, P], F32, tag="ps")
            for ch in range(nch):
                x_t = pix_pool.tile([P, fch], F32, tag="x")
                nc.sync.dma_start(out=x_t, in_=x_in[pl, :, ch * fch:(ch + 1) * fch])

                # idx = int16(255x - 0.5): the int output conversion lands on
                # floor(255x) for non-integer 255x (ties: host-corrected).
                idx = tmp_pool.tile([P, fch], I16, tag="idx")
                nc.vector.tensor_scalar(idx, x_t, 255.0, -0.5, ALU.mult, ALU.add)
                lo = tmp_pool.tile([P, fch], I16, tag="lo")
                nc.vector.tensor_scalar(lo, idx, 15, None, ALU.bitwise_and)

                # him: stationary operand, pack-interleaved (contiguous lhsT
                # slabs). DVE writes are 8-elem runs; ScalarE takes n_act rows
                # as one-pass Sign (exact +-1). Row 0 is the prewritten ones.
                him = him_ab[(pl * nch + ch) % 2]
                idx_r = idx.rearrange("p (s t) -> p s t", t=PACK)
                for j in range(1, 16 - n_act):
                    nc.vector.tensor_scalar(
                        him[:, :, j, :], idx_r, 16.0 * j - 0.5, None, ALU.is_ge
                    )
                for j in range(16 - n_act, 16):
                    nc.scalar.activation(
                        him[:, :, j, :], idx_r, ACTF.Sign,
                        bias=sign_bias[j][:, 0:1], scale=1.0,
                    )

                # lom: moving operand, bin-major (dense [128, fch] writes,
                # DVE 4x perf mode); strided rhs APs are accepted. Row 0 is
                # the prewritten ones.
                lom = lom_ab[(pl * nch + ch) % 2]
                for i in range(1, 16):
                    nc.vector.tensor_scalar(
                        lom[:, i, :], lo, i - 0.5, None, ALU.is_ge
                    )

                for s in range(npack):
                    lhsT = him[:, s].rearrange("p j t -> p (j t)")
                    rhs = lom[:, :, s * PACK:(s + 1) * PACK]
                    nc.tensor.matmul(
                        ps,
                        lhsT,
                        rhs,
                        start=(ch == 0 and s == 0),
                        stop=(ch == nch - 1 and s == npack - 1),
                    )

            # PSUM->SBUF on ScalarE: the copy waits ~1.6us for the plane's
            # matmul chain to drain; keep that stall off the DVE critical path.
            hist_sb = out_pool.tile([P, P], F32, tag="hist")
            nc.scalar.activation(hist_sb, ps, ACTF.Copy)
            nc.sync.dma_start(out=h_out[pl], in_=hist_sb)

    nc.finalize()
    return nc


def _get_nc():
    if "nc" not in _cache:
        _cache["nc"] = build_kernel()
    return _cache["nc"]


def shard_inputs(hist1: np.ndarray, hist2: np.ndarray):
    """Build per-core input maps: core i gets images [4i, 4i+4) of both tensors."""
    in_maps = []
    for i in range(NCORES):
        sl1 = hist1[i * IMGS:(i + 1) * IMGS]  # [4, 3, 512, 512]
        sl2 = hist2[i * IMGS:(i + 1) * IMGS]
        x = np.concatenate(
            [
                np.ascontiguousarray(sl1).reshape(IMGS * C, P, FREE),
                np.ascontiguousarray(sl2).reshape(IMGS * C, P, FREE),
            ],
            axis=0,
        )  # [24, 128, 2048]
        in_maps.append({"x": np.ascontiguousarray(x, dtype=np.float32)})
    return in_maps


def _recovery_mats(n_act=N_ACT_HI):
    """U (him rows) and V (lom rows): S = U N V^T -> N = Uinv S Vinv^T.

    Cumulative rows U[j,a] = 1[a>=j]; ScalarE rows are sign-form 2*1[a>=j]-1.
    """
    a = np.arange(16)
    U = (a[None, :] >= np.arange(16)[:, None]).astype(np.float64)
    U[16 - n_act:, :] = 2.0 * U[16 - n_act:, :] - 1.0
    V = (a[None, :] >= np.arange(16)[:, None]).astype(np.float64)
    return np.linalg.inv(U), np.linalg.inv(V)


_UINV, _VINV = _recovery_mats()


def hist2d_from_raw(raw: np.ndarray) -> np.ndarray:
    """raw: [..., 128, 128] PSUM accumulators -> [..., 256] histograms.

    PSUM row m = 8*j + t, col n = 8*i + t'; valid sums live on the t == t'
    diagonals: S[j, i] = sum_t raw[8j+t, 8i+t].  Then N = Uinv S Vinv^T.
    """
    lead = raw.shape[:-2]
    r = raw.reshape(lead + (16, PACK, 16, PACK)).astype(np.float64)
    S = np.einsum("...jtit->...ji", r)
    N = np.einsum("ja,...ab,ib->...ji", _UINV, S, _VINV)
    N = np.rint(N)
    return N.reshape(lead + (NBINS,))


def fixup_hist(hist: np.ndarray, plane_x: np.ndarray) -> None:
    """Correct the RNE tie cases in-place so counts match exact floor binning.

    Device semantics: for z = fl(255*x) exactly an odd integer k, the RNE tie
    binned the pixel at k-1 instead of k. All other pixels are binned exactly.
    """
    z = plane_x.astype(np.float32) * np.float32(255.0)
    zf = z[z == np.floor(z)]
    if zf.size == 0:
        return
    k = zf.astype(np.int64)
    odd = k[k % 2 == 1]
    for kk, cnt in zip(*np.unique(odd, return_counts=True)):
        hist[kk - 1] -= cnt
        hist[kk] += cnt


def finish_on_host(per_core_hists: list) -> np.ndarray:
    """per_core_hists: NCORES arrays [24, 256] -> scalar chi-square loss."""
    h = np.stack(per_core_hists)  # [8, 24, 256]
    h = h.reshape(NCORES, 2, IMGS, C, NBINS)
    counts1 = h[:, 0].reshape(B, C * NBINS)  # [32, 768]
    counts2 = h[:, 1].reshape(B, C * NBINS)
    n = float(C * H * W)
    h1 = counts1 / n
    h2 = counts2 / n
    chi = np.sum((h1 - h2) ** 2 / (h1 + h2 + BIAS), axis=1)
    return np.array(np.mean(chi), dtype=np.float32)


def kernel(hist1: np.ndarray, hist2: np.ndarray) -> np.ndarray:
    hist1 = np.asarray(hist1, dtype=np.float32)
    hist2 = np.asarray(hist2, dtype=np.float32)
    nc = _get_nc()
    in_maps = shard_inputs(hist1, hist2)
    res = run_bass_kernel_spmd(nc, in_maps, list(range(NCORES)))
    per_core = []
    for i in range(NCORES):
        hists = hist2d_from_raw(res.results[i]["h"])  # [24, 256]
        for pl in range(PLANES):
            fixup_hist(hists[pl], in_maps[i]["x"][pl])
        per_core.append(hists)
    return finish_on_host(per_core)


if __name__ == "__main__":
    rng = np.random.default_rng(0)
    h1 = rng.random((B, C, H, W), dtype=np.float32)
    h2 = rng.random((B, C, H, W), dtype=np.float32)
    out = kernel(h1, h2)
    print("kernel output:", out)


# revision 20
# speedup vs baseline: 5.9997x; 1.0021x over previous
"""Trainium2 Bass kernel for nn_ChiSquareLoss (histogram binning + chi-square).

Strategy (pure data parallel across 8 NeuronCores, 4 images/core):
  - Each core receives 24 "planes" of 512x512 fp32 pixels in [0,1):
    4 images x 3 channels x 2 input tensors, laid out as [24, 128, 2048].
  - Per plane, a 256-bin histogram factored as 16 hi x 16 lo bins, with
    CUMULATIVE (is_ge) factors instead of one-hot:
      idx = int16(255*x - 0.5)   (output-conversion rounding == floor for
            non-integer 255x; rare integer-255x ties corrected on host)
      lo  = idx & 15             (int16 bitwise_and)
      him[j] = (idx >= 16j - .5)           j=0..15   (cumulative hi factor)
      lom[i] = (lo  >= i - .5)             i=0..15   (cumulative lo factor)
    A few him rows are computed on ScalarE as sign(idx - 16j + .5) in
    {-1,+1} (one ACTIVATE pass each); sign = 2*is_ge - 1 keeps the factor
    matrix invertible, and the host solves the 16x16 system exactly.
  - Layouts: him (stationary operand) is pack-interleaved [P, npack, 16, 8]
    so each matmul lhsT slab is a contiguous 128-wide region; lom (moving
    operand) is bin-major [P, 16, fch] so every DVE mask write is a dense
    [128, fch] region (4x perf mode) -- the compiler accepts strided APs
    for the moving operand only.
  - S[j,i] = sum_pixels him[j]*lom[i] via TensorE outer-product matmuls,
    8 pixel-columns packed per [128,128] bf16 matmul accumulated in PSUM;
    the 8 stride-8 diagonal blocks hold S.
  - Host: S = U N V^T with known invertible U, V; recover true counts N,
    assemble [32, 768] histograms, finish chi-square + mean in float64.
"""

import sys

if "/opt/trn_rl_repo" not in sys.path:
    sys.path.insert(0, "/opt/trn_rl_repo")

from contextlib import ExitStack

import numpy as np

import concourse.bacc as bacc
import concourse.bass as bass
import concourse.tile as tile
from concourse import mybir
from concourse.bass_utils import run_bass_kernel_spmd

ALU = mybir.AluOpType
ACTF = mybir.ActivationFunctionType
F32 = mybir.dt.float32
BF16 = mybir.dt.bfloat16
I16 = mybir.dt.int16

B, C, H, W = 32, 3, 512, 512
NCORES = 8
IMGS = B // NCORES            # images per core
PLANES = IMGS * C * 2         # 24 planes per core (hist1 planes then hist2 planes)
P = 128                       # SBUF partitions
FREE = (H * W) // P           # 2048 pixel columns per plane
FCH = 1024                    # free-dim chunk size
NCH = FREE // FCH
PACK = 8                      # pixel columns packed per matmul
NBINS = 256
BIAS = 1e-10

N_ACT_HI = 8                  # him rows on ScalarE via one-pass Sign
N_GPS_LO = 0                  # lom rows on GPSIMD is_ge (port-shared with DVE:
                              # n_gps=3 measured 5x WORSE, 2.6ms -- keep 0)

_cache = {}


def build_kernel(planes=PLANES, free=FREE, fch=FCH, n_act=N_ACT_HI,
                 n_gps=N_GPS_LO):
    nc = bacc.Bacc()
    x_in = nc.declare_dram_parameter("x", [planes, P, free], F32, isOutput=False)
    h_out = nc.declare_dram_parameter("h", [planes, P, P], F32, isOutput=True)

    nch = free // fch
    npack = fch // PACK

    with ExitStack() as ctx:
        tc = ctx.enter_context(tile.TileContext(nc))
        const_pool = ctx.enter_context(tc.tile_pool(name="const", bufs=1))
        pix_pool = ctx.enter_context(tc.tile_pool(name="pix", bufs=3))
        tmp_pool = ctx.enter_context(tc.tile_pool(name="tmp", bufs=2))
        mask_pool = ctx.enter_context(tc.tile_pool(name="mask", bufs=1))
        psum_pool = ctx.enter_context(tc.tile_pool(name="ps", bufs=8, space="PSUM"))
        out_pool = ctx.enter_context(tc.tile_pool(name="hout", bufs=4))

        sign_bias = {}
        for j in range(16 - n_act, 16):
            t = const_pool.tile([P, 1], F32, tag=f"sb{j}")
            nc.vector.memset(t, 0.5 - 16.0 * j)
            sign_bias[j] = t

        # Manually ping-ponged mask buffers. Row 0 of each factor is the
        # all-ones cumulative row (is_ge threshold below range): its value
        # never changes, so it is written once here and never rewritten --
        # saves 2 DVE mask ops per chunk.
        him_ab, lom_ab = [], []
        for h in range(2):
            him_t = mask_pool.tile([P, npack, 16, PACK], BF16, tag=f"him{h}")
            lom_t = mask_pool.tile([P, 16, fch], BF16, tag=f"lom{h}")
            nc.vector.memset(him_t[:, :, 0, :], 1.0)
            nc.vector.memset(lom_t[:, 0, :], 1.0)
            him_ab.append(him_t)
            lom_ab.append(lom_t)

        for pl in range(planes):
            ps = psum_pool.tile([P, P], F32, tag="ps")
            for ch in range(nch):
                x_t = pix_pool.tile([P, fch], F32, tag="x")
                nc.sync.dma_start(out=x_t, in_=x_in[pl, :, ch * fch:(ch + 1) * fch])

                # idx = int16(255x - 0.5): the int output conversion lands on
                # floor(255x) for non-integer 255x (ties: host-corrected).
                idx = tmp_pool.tile([P, fch], I16, tag="idx")
                nc.vector.tensor_scalar(idx, x_t, 255.0, -0.5, ALU.mult, ALU.add)
                lo = tmp_pool.tile([P, fch], I16, tag="lo")
                nc.vector.tensor_scalar(lo, idx, 15, None, ALU.bitwise_and)

                # him: stationary operand, pack-interleaved (contiguous lhsT
                # slabs). DVE writes are 8-elem runs; ScalarE takes n_act rows
                # as one-pass Sign (exact +-1). Row 0 is the prewritten ones.
                him = him_ab[(pl * nch + ch) % 2]
                idx_r = idx.rearrange("p (s t) -> p s t", t=PACK)
                for j in range(1, 16 - n_act):
                    nc.vector.tensor_scalar(
                        him[:, :, j, :], idx_r, 16.0 * j - 0.5, None, ALU.is_ge
                    )
                for j in range(16 - n_act, 16):
                    nc.scalar.activation(
                        him[:, :, j, :], idx_r, ACTF.Sign,
                        bias=sign_bias[j][:, 0:1], scale=1.0,
                    )

                # lom: moving operand, bin-major (dense [128, fch] writes,
                # DVE 4x perf mode); strided rhs APs are accepted. Row 0 is
                # the prewritten ones.
                lom = lom_ab[(pl * nch + ch) % 2]
                for i in range(1, 16):
                    nc.vector.tensor_scalar(
                        lom[:, i, :], lo, i - 0.5, None, ALU.is_ge
                    )

                for s in range(npack):
                    lhsT = him[:, s].rearrange("p j t -> p (j t)")
                    rhs = lom[:, :, s * PACK:(s + 1) * PACK]
                    nc.tensor.matmul(
                        ps,
                        lhsT,
                        rhs,
                        start=(ch == 0 and s == 0),
                        stop=(ch == nch - 1 and s == npack - 1),
                    )

            # PSUM->SBUF on ScalarE: the copy waits ~1.6us for the plane's
            # matmul chain to drain; keep that stall off the DVE critical path.
            hist_sb = out_pool.tile([P, P], F32, tag="hist")
            nc.scalar.activation(hist_sb, ps, ACTF.Copy)
            nc.sync.dma_start(out=h_out[pl], in_=hist_sb)

    nc.finalize()
    return nc


def _get_nc():
    if "nc" not in _cache:
        _cache["nc"] = build_kernel()
    return _cache["nc"]


def shard_inputs(hist1: np.ndarray, hist2: np.ndarray):
    """Build per-core input maps: core i gets images [4i, 4i+4) of both tensors."""
    in_maps = []
    for i in range(NCORES):
        sl1 = hist1[i * IMGS:(i + 1) * IMGS]  # [4, 3, 512, 512]
        sl2 = hist2[i * IMGS:(i + 1) * IMGS]
        x = np.concatenate(
            [
                np.ascontiguousarray(sl1).reshape(IMGS * C, P, FREE),
                np.ascontiguousarray(sl2).reshape(IMGS * C, P, FREE),
            ],
            axis=0,
        )  # [24, 128, 2048]
        in_maps.append({"x": np.ascontiguousarray(x, dtype=np.float32)})
    return in_maps


def _recovery_mats(n_act=N_ACT_HI):
    """U (him rows) and V (lom rows): S = U N V^T -> N = Uinv S Vinv^T.

    Cumulative rows U[j,a] = 1[a>=j]; ScalarE rows are sign-form 2*1[a>=j]-1.
    """
    a = np.arange(16)
    U = (a[None, :] >= np.arange(16)[:, None]).astype(np.float64)
    U[16 - n_act:, :] = 2.0 * U[16 - n_act:, :] - 1.0
    V = (a[None, :] >= np.arange(16)[:, None]).astype(np.float64)
    return np.linalg.inv(U), np.linalg.inv(V)


_UINV, _VINV = _recovery_mats()


def hist2d_from_raw(raw: np.ndarray) -> np.ndarray:
    """raw: [..., 128, 128] PSUM accumulators -> [..., 256] histograms.

    PSUM row m = 8*j + t, col n = 8*i + t'; valid sums live on the t == t'
    diagonals: S[j, i] = sum_t raw[8j+t, 8i+t].  Then N = Uinv S Vinv^T.
    """
    lead = raw.shape[:-2]
    r = raw.reshape(lead + (16, PACK, 16, PACK)).astype(np.float64)
    S = np.einsum("...jtit->...ji", r)
    N = np.einsum("ja,...ab,ib->...ji", _UINV, S, _VINV)
    N = np.rint(N)
    return N.reshape(lead + (NBINS,))


def fixup_hist(hist: np.ndarray, plane_x: np.ndarray) -> None:
    """Correct the RNE tie cases in-place so counts match exact floor binning.

    Device semantics: for z = fl(255*x) exactly an odd integer k, the RNE tie
    binned the pixel at k-1 instead of k. All other pixels are binned exactly.
    """
    z = plane_x.astype(np.float32) * np.float32(255.0)
    zf = z[z == np.floor(z)]
    if zf.size == 0:
        return
    k = zf.astype(np.int64)
    odd = k[k % 2 == 1]
    for kk, cnt in zip(*np.unique(odd, return_counts=True)):
        hist[kk - 1] -= cnt
        hist[kk] += cnt


def finish_on_host(per_core_hists: list) -> np.ndarray:
    """per_core_hists: NCORES arrays [24, 256] -> scalar chi-square loss."""
    h = np.stack(per_core_hists)  # [8, 24, 256]
    h = h.reshape(NCORES, 2, IMGS, C, NBINS)
    counts1 = h[:, 0].reshape(B, C * NBINS)  # [32, 768]
    counts2 = h[:, 1].reshape(B, C * NBINS)
    n = float(C * H * W)
    h1 = counts1 / n
    h2 = counts2 / n
    chi = np.sum((h1 - h2) ** 2 / (h1 + h2 + BIAS), axis=1)
    return np.array(np.mean(chi), dtype=np.float32)


def kernel(hist1: np.ndarray, hist2: np.ndarray) -> np.ndarray:
    hist1 = np.asarray(hist1, dtype=np.float32)
    hist2 = np.asarray(hist2, dtype=np.float32)
    nc = _get_nc()
    in_maps = shard_inputs(hist1, hist2)
    res = run_bass_kernel_spmd(nc, in_maps, list(range(NCORES)))
    per_core = []
    for i in range(NCORES):
        hists = hist2d_from_raw(res.results[i]["h"])  # [24, 256]
        for pl in range(PLANES):
            fixup_hist(hists[pl], in_maps[i]["x"][pl])
        per_core.append(hists)
    return finish_on_host(per_core)


if __name__ == "__main__":
    rng = np.random.default_rng(0)
    h1 = rng.random((B, C, H, W), dtype=np.float32)
    h2 = rng.random((B, C, H, W), dtype=np.float32)
    out = kernel(h1, h2)
    print("kernel output:", out)
